# revision 23
# baseline (speedup 1.0000x reference)
"""CARAFE kernel for Trainium2 (8 NeuronCores, batch-parallel), v2.

Reference computation per image:
  R = relu(conv1x1(x, w_compress, b_compress))          [48, 128, 128]
  E = conv3x3(R, w_encoder, b_encoder, pad=1)           [100, 128, 128]
  Y = softmax over k of E.reshape(4, 25, H, W)          (s, k, h, w)
  out[s,c,h,w] = sum_k Y[s,k,h,w] * xpad[c, h+dy, w+dx] (k=(dy,dx), 5x5, pad 2)
  pixel-shuffle: out_ref[s*16 + c//4, 2h + (c//2)%2, 2w + c%2] = out[s,c,h,w]

v2 design (vs v1):
  - bf16 everywhere: PE matmuls at 1 cyc/row (vs 4 for fp32), DVE
    tensor_tensor at 2x, all DMA bytes halved.  PSUM accumulation stays
    fp32, exp runs on fp32 PSUM logits.
  - The five dy-shifted pixel-major copies of x (needed because compute
    engines cannot shift partitions) are built on the HOST as a pure
    layout transform and shipped as one [128, 5*C*(W+4)] bf16 input --
    one contiguous DMA instead of 40 SBUF->SBUF copies with 272B
    descriptors (which dominated v1: 328K DMA packets).
  - Softmax normalization folded in per conv block: Z = sones @ F on PE,
    1/Z on DVE, broadcast back over the 25 taps via a second tiny matmul,
    one in-place multiply.  No F DRAM round-trip, no per-(s,w) epilogue.
  - Output leaves the device as raw [s, h, (c,w)] bf16 (4 contiguous 2MB
    DMAs); pixel-shuffle + fp32 cast happen on the host.
"""

import sys

import numpy as np

sys.path.insert(0, "/opt/trn_rl_repo")

import ml_dtypes

import concourse.bass as bass
import concourse.mybir as mybir
import concourse.tile as tile
from concourse import bacc

F32 = mybir.dt.float32
# fp16 (not bf16): the 25-term sequential accumulation in the patch sum
# needs the 10-bit mantissa — bf16's 7 bits put rel-err right at the 2e-2
# gate; fp16 lands at ~2e-3.  Values are O(1-5), far from fp16 range limits.
BF16 = mybir.dt.float16
BF_NP = np.float16

H = 128
W = 128
C = 64
M = 48  # compressed channels
S2 = 4  # scale_factor**2
K2 = 25  # k_up**2
SK = 100
HW = H * W
WPAD = W + 4  # w-padded pixel-major buffers
CW = C * WPAD  # 8448, free elems per (dy) plane
N_CORES = 8
NBLK = HW // 512  # 32 conv blocks of 512 pixels


def _ap(t, extra_off, dims):
    """Raw AP on a tile handle `t` with free-offset `extra_off` (elements)
    and explicit [step, count] dims (dims[0] is the partition dim)."""
    base = t[:]
    return bass.AP(tensor=base.tensor, offset=base.offset + extra_off, ap=dims)


class _Pool:
    """Manually scoped tile pool."""

    def __init__(self, tc, **kw):
        self._cm = tc.tile_pool(**kw)
        self.pool = self._cm.__enter__()
        self._n = 0

    def tile(self, *a, tag=None, **kw):
        self._n += 1
        t = tag or f"t{self._n}"
        return self.pool.tile(*a, tag=t, name=t, **kw)

    def close(self):
        self._cm.__exit__(None, None, None)


def build_program():
    nc = bacc.Bacc("TRN2", target_bir_lowering=False, debug=False)

    xc = nc.dram_tensor("xc", [C + 1, HW], BF16, kind="ExternalInput")
    xt5d = nc.dram_tensor("xt5", [128, 5 * CW], BF16, kind="ExternalInput")
    w1t = nc.dram_tensor("w1t", [C + 1, M], BF16, kind="ExternalInput")
    wet = nc.dram_tensor("wet", [M + 1, 9 * SK], BF16, kind="ExternalInput")
    sones = nc.dram_tensor("sones", [SK, S2], BF16, kind="ExternalInput")
    sonesT = nc.dram_tensor("sonesT", [S2, SK], BF16, kind="ExternalInput")
    identd = nc.dram_tensor("ident", [SK, SK], BF16, kind="ExternalInput")
    onesr = nc.dram_tensor("onesr", [1, 130 * 130], BF16, kind="ExternalInput")
    out = nc.dram_tensor("out", [S2, H, C * W], BF16, kind="ExternalOutput")

    with tile.TileContext(nc) as tc:
        cp = _Pool(tc, name="consts", bufs=1)
        w1t_sb = cp.tile([C + 1, M], BF16)
        nc.sync.dma_start(w1t_sb[:], w1t.ap())
        wet_sb = cp.tile([M + 1, 9 * SK], BF16)
        nc.sync.dma_start(wet_sb[:], wet.ap())
        sones_sb = cp.tile([SK, S2], BF16)
        nc.sync.dma_start(sones_sb[:], sones.ap())
        sonesT_sb = cp.tile([S2, SK], BF16)
        nc.sync.dma_start(sonesT_sb[:], sonesT.ap())
        ident_sb = cp.tile([SK, SK], BF16)
        nc.sync.dma_start(ident_sb[:], identd.ap())

        pp = _Pool(tc, name="persist", bufs=1)
        xt5 = pp.tile([128, 5 * CW], BF16)
        nc.sync.dma_start(xt5[:], xt5d.ap())
        fr = pp.tile([128, SK * W], BF16)

        p2 = _Pool(tc, name="fnorm", bufs=1)
        f_norm = p2.tile([SK, HW], BF16)

        # ---- conv1x1 -> relu -> R_pad (full image, 1-halo borders) ----
        p3 = _Pool(tc, name="rpad", bufs=1)
        r_pad = p3.tile([M + 1, 130 * 130], BF16)
        nc.gpsimd.memset(r_pad[:], 0.0)
        nc.sync.dma_start(
            _ap(r_pad, M * 130 * 130, [[130 * 130, 1], [1, 130 * 130]]), onesr.ap()
        )

        p4 = _Pool(tc, name="xb", bufs=2)
        psA = _Pool(tc, name="psA", bufs=2, space="PSUM")
        CHUNK = 8  # conv1x1 blocks per x chunk-load
        for jc in range(NBLK // CHUNK):
            xb = p4.tile([C + 1, 512 * CHUNK], BF16, tag="xb")
            nc.sync.dma_start(
                xb[:], xc.ap()[:, jc * 512 * CHUNK : (jc + 1) * 512 * CHUNK]
            )
            for ji in range(CHUNK):
                j = jc * CHUNK + ji
                ps1 = psA.tile([M, 512], F32, tag="ps1")
                nc.tensor.matmul(
                    ps1[:],
                    w1t_sb[:],
                    xb[:, ji * 512 : (ji + 1) * 512],
                    start=True,
                    stop=True,
                )
                nc.scalar.activation(
                    _ap(
                        r_pad,
                        (1 + 4 * j) * 130 + 1,
                        [[130 * 130, M], [130, 4], [1, W]],
                    ),
                    ps1[:],
                    mybir.ActivationFunctionType.Relu,
                )
        psA.close()
        p4.close()

        # ---- conv3x3 -> exp -> normalized F (channel-major, bf16) ----
        psB = _Pool(tc, name="psB", bufs=4, space="PSUM")
        psC = _Pool(tc, name="psC", bufs=2, space="PSUM")
        psD = _Pool(tc, name="psD", bufs=2, space="PSUM")
        rzp = _Pool(tc, name="rz", bufs=2)
        for j in range(NBLK):
            ps2 = psB.tile([SK, 512], F32, tag="ps2")
            for t in range(9):
                ty, tx = divmod(t, 3)
                nc.tensor.matmul(
                    ps2[:],
                    wet_sb[:, t * SK : (t + 1) * SK],
                    _ap(r_pad, (4 * j + ty) * 130 + tx, [[130 * 130, M + 1], [130, 4], [1, W]]),
                    start=(t == 0),
                    stop=(t == 8),
                )
            fblk = f_norm[:, j * 512 : (j + 1) * 512]
            nc.scalar.activation(fblk, ps2[:], mybir.ActivationFunctionType.Exp)
            psz = psC.tile([S2, 512], F32, tag="psz")
            nc.tensor.matmul(psz[:], sones_sb[:], fblk, start=True, stop=True)
            rz32 = rzp.tile([S2, 512], F32, tag="rz32")
            nc.vector.reciprocal_approx_fast(rz32[:], psz[:])
            rz = rzp.tile([S2, 512], BF16, tag="rz")
            nc.scalar.copy(rz[:], rz32[:])
            zb = psD.tile([SK, 512], F32, tag="zb")
            nc.tensor.matmul(zb[:], sonesT_sb[:], rz[:], start=True, stop=True)
            nc.vector.tensor_mul(fblk, fblk, zb[:])
        rzp.close()
        psD.close()
        psC.close()
        psB.close()
        p3.close()

        # ---- F^T transposes -> FR [128(h), (sk, w)] ----
        # 8 per-column transposes share one PSUM bank, then a single strided
        # ScalarE copy moves all 8 columns into FR (vs 128 tiny copies).
        psF = _Pool(tc, name="psF", bufs=4, space="PSUM")
        FTG = 8
        for w0 in range(0, W, FTG):
            pst = psF.tile([128, FTG * SK], BF16, tag="pst")
            for wi in range(FTG):
                nc.tensor.transpose(
                    pst[:, wi * SK : (wi + 1) * SK],
                    _ap(f_norm, w0 + wi, [[HW, SK], [W, H]]),
                    ident_sb[:],
                )
            nc.scalar.copy(
                _ap(fr, w0, [[SK * W, 128], [1, FTG], [W, SK]]),
                _ap(pst, 0, [[FTG * SK, 128], [SK, FTG], [1, SK]]),
            )
        psF.close()
        p2.close()

        # ---- per-pixel patch sum; out[s] = [128(h), (c, w)] ----
        # DVE owns taps 0..N_DVE-1; the otherwise-idle GPSIMD engine owns the
        # rest in its own accumulator chain; one DVE add merges them.
        N_GP = 5  # taps 20..24 run on GPSIMD
        pacc = _Pool(tc, name="acc", bufs=1)
        ptmp = _Pool(tc, name="tmp", bufs=2)
        pgac = _Pool(tc, name="gacc", bufs=1)
        pgtm = _Pool(tc, name="gtmp", bufs=1)

        def tap_aps(s, k, t):
            dy, dx = k // 5 - 2, k % 5 - 2
            in0 = _ap(
                xt5, (dy + 2) * CW + 2 + dx, [[5 * CW, 128], [WPAD, C], [1, W]]
            )
            in1 = _ap(fr, (s * K2 + k) * W, [[SK * W, 128], [0, C], [1, W]])
            dst3 = _ap(t, 0, [[C * W, 128], [W, C], [1, W]])
            return in0, in1, dst3

        for s in range(S2):
            acc = pacc.tile([128, C * W], BF16, tag="acc")
            gacc = pgac.tile([128, C * W], BF16, tag="gacc")
            for k in range(25 - N_GP, 25):
                in0, in1, dst3 = tap_aps(s, k, gacc)
                if k == 25 - N_GP:
                    nc.gpsimd.tensor_mul(dst3, in0, in1)
                else:
                    gtmp = pgtm.tile([128, C * W], BF16, tag="gtmp")
                    _, _, t3 = tap_aps(s, k, gtmp)
                    nc.gpsimd.tensor_mul(t3, in0, in1)
                    nc.gpsimd.tensor_add(gacc[:], gacc[:], gtmp[:])
            for k in range(25 - N_GP):
                in0, in1, dst3 = tap_aps(s, k, acc)
                if k == 0:
                    nc.vector.tensor_mul(dst3, in0, in1)
                else:
                    tmp = ptmp.tile([128, C * W], BF16, tag="tmp")
                    _, _, t3 = tap_aps(s, k, tmp)
                    nc.vector.tensor_mul(t3, in0, in1)
                    nc.vector.tensor_add(acc[:], acc[:], tmp[:])
            nc.vector.tensor_add(acc[:], acc[:], gacc[:])
            nc.sync.dma_start(
                bass.AP(tensor=out, offset=s * H * C * W, ap=[[C * W, 128], [1, C * W]]),
                acc[:],
            )
        pgtm.close()
        pgac.close()
        ptmp.close()
        pacc.close()
        pp.close()
        cp.close()
    nc.compile()
    return nc


def host_inputs(x_img, w_compress, b_compress, w_encoder, b_encoder):
    """Per-core input map for one image [C, H, W] (all bf16)."""
    x_img = np.asarray(x_img, np.float32)
    xc = np.concatenate(
        [x_img.reshape(C, HW), np.ones((1, HW), np.float32)], axis=0
    ).astype(BF_NP)
    # pixel-major, w-padded, 5 dy-shifted planes: xt5[h, dy, c, wp]
    #   = xpad[c, h + dy, wp]  (xpad has pad 2 on h and w)
    xpad = np.pad(x_img, ((0, 0), (2, 2), (2, 2))).astype(BF_NP)
    xt5 = np.stack([xpad[:, dy : dy + H, :] for dy in range(5)], axis=0)
    xt5 = np.ascontiguousarray(xt5.transpose(2, 0, 1, 3)).reshape(128, 5 * CW)
    w1t = np.concatenate(
        [w_compress[:, :, 0, 0].T, b_compress[None, :]], axis=0
    ).astype(BF_NP)
    wetm = np.zeros((M + 1, 9, SK), np.float32)
    for ty in range(3):
        for tx in range(3):
            wetm[:M, ty * 3 + tx, :] = w_encoder[:, :, ty, tx].T
    wetm[M, 4, :] = b_encoder
    son = np.zeros((SK, S2), np.float32)
    for s in range(S2):
        son[s * K2 : (s + 1) * K2, s] = 1.0
    return {
        "xc": xc,
        "xt5": xt5,
        "w1t": w1t,
        "wet": wetm.reshape(M + 1, 9 * SK).astype(BF_NP),
        "sones": son.astype(BF_NP),
        "sonesT": np.ascontiguousarray(son.T).astype(BF_NP),
        "ident": np.eye(SK, dtype=BF_NP),
        "onesr": np.ones((1, 130 * 130), BF_NP),
    }


def _unshuffle(dev_out):
    """[S2, H, C*W] bf16 -> [64, 256, 256] fp32 pixel-shuffled output."""
    a = np.asarray(dev_out).reshape(S2, H, 16, 2, 2, W)  # s, h, c4, c2, c1, w
    a = a.transpose(0, 2, 1, 3, 5, 4)  # s, c4, h, c2, w, c1
    return np.ascontiguousarray(a).reshape(C, 2 * H, 2 * W).astype(np.float32)


_CACHE = {}


def kernel(x, w_compress, b_compress, w_encoder, b_encoder):
    x = np.asarray(x, np.float32)
    if "nc" not in _CACHE:
        _CACHE["nc"] = build_program()
    nc = _CACHE["nc"]
    in_maps = [
        host_inputs(
            x[i],
            np.asarray(w_compress, np.float32),
            np.asarray(b_compress, np.float32),
            np.asarray(w_encoder, np.float32),
            np.asarray(b_encoder, np.float32),
        )
        for i in range(N_CORES)
    ]
    from concourse.bass_utils import run_bass_kernel_spmd

    res = run_bass_kernel_spmd(nc, in_maps, core_ids=list(range(N_CORES)))
    return np.stack(
        [_unshuffle(res.results[i]["out"]) for i in range(N_CORES)], axis=0
    )


# revision 31
# speedup vs baseline: 1.3375x; 1.3375x over previous
"""CARAFE kernel for Trainium2 (8 NeuronCores, batch-parallel), v2.

Reference computation per image:
  R = relu(conv1x1(x, w_compress, b_compress))          [48, 128, 128]
  E = conv3x3(R, w_encoder, b_encoder, pad=1)           [100, 128, 128]
  Y = softmax over k of E.reshape(4, 25, H, W)          (s, k, h, w)
  out[s,c,h,w] = sum_k Y[s,k,h,w] * xpad[c, h+dy, w+dx] (k=(dy,dx), 5x5, pad 2)
  pixel-shuffle: out_ref[s*16 + c//4, 2h + (c//2)%2, 2w + c%2] = out[s,c,h,w]

v2 design (vs v1):
  - bf16 everywhere: PE matmuls at 1 cyc/row (vs 4 for fp32), DVE
    tensor_tensor at 2x, all DMA bytes halved.  PSUM accumulation stays
    fp32, exp runs on fp32 PSUM logits.
  - The five dy-shifted pixel-major copies of x (needed because compute
    engines cannot shift partitions) are built on the HOST as a pure
    layout transform and shipped as one [128, 5*C*(W+4)] bf16 input --
    one contiguous DMA instead of 40 SBUF->SBUF copies with 272B
    descriptors (which dominated v1: 328K DMA packets).
  - Softmax normalization folded in per conv block: Z = sones @ F on PE,
    1/Z on DVE, broadcast back over the 25 taps via a second tiny matmul,
    one in-place multiply.  No F DRAM round-trip, no per-(s,w) epilogue.
  - Output leaves the device as raw [s, h, (c,w)] bf16 (4 contiguous 2MB
    DMAs); pixel-shuffle + fp32 cast happen on the host.
"""

import sys

import numpy as np

sys.path.insert(0, "/opt/trn_rl_repo")

import ml_dtypes

import concourse.bass as bass
import concourse.mybir as mybir
import concourse.tile as tile
from concourse import bacc

F32 = mybir.dt.float32
# fp16 (not bf16): the 25-term sequential accumulation in the patch sum
# needs the 10-bit mantissa — bf16's 7 bits put rel-err right at the 2e-2
# gate; fp16 lands at ~2e-3.  Values are O(1-5), far from fp16 range limits.
BF16 = mybir.dt.float16
BF_NP = np.float16

H = 128
W = 128
C = 64
M = 48  # compressed channels
S2 = 4  # scale_factor**2
K2 = 25  # k_up**2
SK = 100
HW = H * W
WPAD = W + 4  # w-padded pixel-major buffers
CW = C * WPAD  # 8448, free elems per (dy) plane
N_CORES = 8
NBLK = HW // 512  # 32 conv blocks of 512 pixels


def _ap(t, extra_off, dims):
    """Raw AP on a tile handle `t` with free-offset `extra_off` (elements)
    and explicit [step, count] dims (dims[0] is the partition dim)."""
    base = t[:]
    return bass.AP(tensor=base.tensor, offset=base.offset + extra_off, ap=dims)


class _Pool:
    """Manually scoped tile pool."""

    def __init__(self, tc, **kw):
        self._cm = tc.tile_pool(**kw)
        self.pool = self._cm.__enter__()
        self._n = 0

    def tile(self, *a, tag=None, **kw):
        self._n += 1
        t = tag or f"t{self._n}"
        return self.pool.tile(*a, tag=t, name=t, **kw)

    def close(self):
        self._cm.__exit__(None, None, None)


def build_program():
    nc = bacc.Bacc("TRN2", target_bir_lowering=False, debug=False)

    xc = nc.dram_tensor("xc", [C + 1, HW], BF16, kind="ExternalInput")
    xt5d = nc.dram_tensor("xt5", [128, 5 * CW], BF16, kind="ExternalInput")
    w1t = nc.dram_tensor("w1t", [C + 1, M], BF16, kind="ExternalInput")
    wet = nc.dram_tensor("wet", [113, 6 * SK], BF16, kind="ExternalInput")
    sones = nc.dram_tensor("sones", [SK, S2], BF16, kind="ExternalInput")
    sonesT = nc.dram_tensor("sonesT", [S2, SK], BF16, kind="ExternalInput")
    identd = nc.dram_tensor("ident", [SK, SK], BF16, kind="ExternalInput")
    onesr = nc.dram_tensor("onesr", [1, 130 * 130], BF16, kind="ExternalInput")
    out = nc.dram_tensor("out", [S2, H, C * W], BF16, kind="ExternalOutput")

    with tile.TileContext(nc) as tc:
        cp = _Pool(tc, name="consts", bufs=1)
        w1t_sb = cp.tile([C + 1, M], BF16)
        nc.sync.dma_start(w1t_sb[:], w1t.ap())
        wet_sb = cp.tile([113, 6 * SK], BF16)
        nc.sync.dma_start(wet_sb[:], wet.ap())
        sones_sb = cp.tile([SK, S2], BF16)
        nc.sync.dma_start(sones_sb[:], sones.ap())
        sonesT_sb = cp.tile([S2, SK], BF16)
        nc.sync.dma_start(sonesT_sb[:], sonesT.ap())
        ident_sb = cp.tile([SK, SK], BF16)
        nc.sync.dma_start(ident_sb[:], identd.ap())

        pp = _Pool(tc, name="persist", bufs=1)
        xt5 = pp.tile([128, 5 * CW], BF16)
        nc.sync.dma_start(xt5[:], xt5d.ap())
        fr = pp.tile([128, SK * W], BF16)

        p2 = _Pool(tc, name="fnorm", bufs=1)
        f_norm = p2.tile([SK, HW], BF16)

        # ---- conv1x1 -> relu -> R_pad (full image, 1-halo borders) ----
        # Two copies of R on partitions 0..48 (A) and 64..112 (B, shifted one
        # w-column left) let conv3x3 contract taps (ty,0)+(ty,1) in a single
        # K=113 matmul: 6 matmuls per block instead of 9.
        RPF = 130 * 130
        p3 = _Pool(tc, name="rpad", bufs=1)
        r_pad = p3.tile([113, RPF], BF16)
        nc.gpsimd.memset(r_pad[:], 0.0)
        nc.sync.dma_start(
            _ap(r_pad, M * RPF, [[RPF, 1], [1, RPF]]), onesr.ap()
        )
        nc.sync.dma_start(
            _ap(r_pad, (M + 64) * RPF, [[RPF, 1], [1, RPF]]), onesr.ap()
        )

        p4 = _Pool(tc, name="xb", bufs=2)
        psA = _Pool(tc, name="psA", bufs=2, space="PSUM")
        CHUNK = 8  # conv1x1 blocks per x chunk-load
        for jc in range(NBLK // CHUNK):
            xb = p4.tile([C + 1, 512 * CHUNK], BF16, tag="xb")
            nc.sync.dma_start(
                xb[:], xc.ap()[:, jc * 512 * CHUNK : (jc + 1) * 512 * CHUNK]
            )
            for ji in range(CHUNK):
                j = jc * CHUNK + ji
                ps1 = psA.tile([M, 512], F32, tag="ps1")
                nc.tensor.matmul(
                    ps1[:],
                    w1t_sb[:],
                    xb[:, ji * 512 : (ji + 1) * 512],
                    start=True,
                    stop=True,
                )
                nc.scalar.activation(
                    _ap(
                        r_pad,
                        (1 + 4 * j) * 130 + 1,
                        [[RPF, M], [130, 4], [1, W]],
                    ),
                    ps1[:],
                    mybir.ActivationFunctionType.Relu,
                )
                nc.scalar.activation(
                    _ap(
                        r_pad,
                        64 * RPF + (1 + 4 * j) * 130,
                        [[RPF, M], [130, 4], [1, W]],
                    ),
                    ps1[:],
                    mybir.ActivationFunctionType.Relu,
                )
        psA.close()
        p4.close()

        # ---- conv3x3 -> exp -> normalized F (channel-major, bf16) ----
        psB = _Pool(tc, name="psB", bufs=4, space="PSUM")
        psC = _Pool(tc, name="psC", bufs=2, space="PSUM")
        psD = _Pool(tc, name="psD", bufs=2, space="PSUM")
        rzp = _Pool(tc, name="rz", bufs=2)
        # slots: 3 paired (taps (ty,0)+(ty,1), K=113) + 3 single (taps (ty,2), K=49)
        SLOTS = [(0, 113), (1, 113), (2, 113), (0, 49), (1, 49), (2, 49)]
        for j in range(NBLK):
            ps2 = psB.tile([SK, 512], F32, tag="ps2")
            for m, (ty, kk) in enumerate(SLOTS):
                off = (4 * j + ty) * 130 + (0 if kk == 113 else 2)
                nc.tensor.matmul(
                    ps2[:],
                    wet_sb[0:kk, m * SK : (m + 1) * SK],
                    _ap(r_pad, off, [[RPF, kk], [130, 4], [1, W]]),
                    start=(m == 0),
                    stop=(m == len(SLOTS) - 1),
                )
            fblk = f_norm[:, j * 512 : (j + 1) * 512]
            nc.scalar.activation(fblk, ps2[:], mybir.ActivationFunctionType.Exp)
            psz = psC.tile([S2, 512], F32, tag="psz")
            nc.tensor.matmul(psz[:], sones_sb[:], fblk, start=True, stop=True)
            rz32 = rzp.tile([S2, 512], F32, tag="rz32")
            nc.vector.reciprocal_approx_fast(rz32[:], psz[:])
            rz = rzp.tile([S2, 512], BF16, tag="rz")
            nc.scalar.copy(rz[:], rz32[:])
            zb = psD.tile([SK, 512], F32, tag="zb")
            nc.tensor.matmul(zb[:], sonesT_sb[:], rz[:], start=True, stop=True)
            nc.vector.tensor_mul(fblk, fblk, zb[:])
        rzp.close()
        psD.close()
        psC.close()
        psB.close()
        p3.close()

        # ---- F^T transposes -> FR [128(h), (sk, w)] ----
        # 8 per-column transposes share one PSUM bank, then a single strided
        # ScalarE copy moves all 8 columns into FR (vs 128 tiny copies).
        psF = _Pool(tc, name="psF", bufs=4, space="PSUM")
        FTG = 8
        for w0 in range(0, W, FTG):
            pst = psF.tile([128, FTG * SK], BF16, tag="pst")
            for wi in range(FTG):
                nc.tensor.transpose(
                    pst[:, wi * SK : (wi + 1) * SK],
                    _ap(f_norm, w0 + wi, [[HW, SK], [W, H]]),
                    ident_sb[:],
                )
            nc.scalar.copy(
                _ap(fr, w0, [[SK * W, 128], [1, FTG], [W, SK]]),
                _ap(pst, 0, [[FTG * SK, 128], [SK, FTG], [1, SK]]),
            )
        psF.close()
        p2.close()

        # ---- per-pixel patch sum on VectorE; out[s] = [128(h), (c, w)] ----
        pacc = _Pool(tc, name="acc", bufs=2)
        ptmp = _Pool(tc, name="tmp", bufs=2)

        def tap_aps(s, k, t):
            dy, dx = k // 5 - 2, k % 5 - 2
            in0 = _ap(
                xt5, (dy + 2) * CW + 2 + dx, [[5 * CW, 128], [WPAD, C], [1, W]]
            )
            in1 = _ap(fr, (s * K2 + k) * W, [[SK * W, 128], [0, C], [1, W]])
            dst3 = _ap(t, 0, [[C * W, 128], [W, C], [1, W]])
            return in0, in1, dst3

        for s in range(S2):
            acc = pacc.tile([128, C * W], BF16, tag="acc")
            for k in range(25):
                in0, in1, dst3 = tap_aps(s, k, acc)
                if k == 0:
                    nc.vector.tensor_mul(dst3, in0, in1)
                else:
                    tmp = ptmp.tile([128, C * W], BF16, tag="tmp")
                    _, _, t3 = tap_aps(s, k, tmp)
                    nc.vector.tensor_mul(t3, in0, in1)
                    nc.vector.tensor_add(acc[:], acc[:], tmp[:])
            nc.sync.dma_start(
                bass.AP(tensor=out, offset=s * H * C * W, ap=[[C * W, 128], [1, C * W]]),
                acc[:],
            )
        ptmp.close()
        pacc.close()
        pp.close()
        cp.close()
    nc.compile()
    return nc


def host_inputs(x_img, w_compress, b_compress, w_encoder, b_encoder):
    """Per-core input map for one image [C, H, W] (all bf16)."""
    x_img = np.asarray(x_img, np.float32)
    xc = np.concatenate(
        [x_img.reshape(C, HW), np.ones((1, HW), np.float32)], axis=0
    ).astype(BF_NP)
    # pixel-major, w-padded, 5 dy-shifted planes: xt5[h, dy, c, wp]
    #   = xpad[c, h + dy, wp]  (xpad has pad 2 on h and w)
    xpad = np.pad(x_img, ((0, 0), (2, 2), (2, 2))).astype(BF_NP)
    xt5 = np.stack([xpad[:, dy : dy + H, :] for dy in range(5)], axis=0)
    xt5 = np.ascontiguousarray(xt5.transpose(2, 0, 1, 3)).reshape(128, 5 * CW)
    w1t = np.concatenate(
        [w_compress[:, :, 0, 0].T, b_compress[None, :]], axis=0
    ).astype(BF_NP)
    # paired layout: slots 0-2 = taps (ty,0) on rows 0..47 + (ty,1) on rows
    # 64..111; slots 3-5 = single taps (ty,2).  Bias rides the all-ones rows
    # (48 for A, 112 for B) on the center tap (1,1) = slot 1's B half.
    wetm = np.zeros((113, 6, SK), np.float32)
    for ty in range(3):
        wetm[:M, ty, :] = w_encoder[:, :, ty, 0].T
        wetm[64 : 64 + M, ty, :] = w_encoder[:, :, ty, 1].T
        wetm[:M, 3 + ty, :] = w_encoder[:, :, ty, 2].T
    wetm[112, 1, :] = b_encoder
    son = np.zeros((SK, S2), np.float32)
    for s in range(S2):
        son[s * K2 : (s + 1) * K2, s] = 1.0
    return {
        "xc": xc,
        "xt5": xt5,
        "w1t": w1t,
        "wet": wetm.reshape(113, 6 * SK).astype(BF_NP),
        "sones": son.astype(BF_NP),
        "sonesT": np.ascontiguousarray(son.T).astype(BF_NP),
        "ident": np.eye(SK, dtype=BF_NP),
        "onesr": np.ones((1, 130 * 130), BF_NP),
    }


def _unshuffle(dev_out):
    """[S2, H, C*W] bf16 -> [64, 256, 256] fp32 pixel-shuffled output."""
    a = np.asarray(dev_out).reshape(S2, H, 16, 2, 2, W)  # s, h, c4, c2, c1, w
    a = a.transpose(0, 2, 1, 3, 5, 4)  # s, c4, h, c2, w, c1
    return np.ascontiguousarray(a).reshape(C, 2 * H, 2 * W).astype(np.float32)


_CACHE = {}


def kernel(x, w_compress, b_compress, w_encoder, b_encoder):
    x = np.asarray(x, np.float32)
    if "nc" not in _CACHE:
        _CACHE["nc"] = build_program()
    nc = _CACHE["nc"]
    in_maps = [
        host_inputs(
            x[i],
            np.asarray(w_compress, np.float32),
            np.asarray(b_compress, np.float32),
            np.asarray(w_encoder, np.float32),
            np.asarray(b_encoder, np.float32),
        )
        for i in range(N_CORES)
    ]
    from concourse.bass_utils import run_bass_kernel_spmd

    res = run_bass_kernel_spmd(nc, in_maps, core_ids=list(range(N_CORES)))
    return np.stack(
        [_unshuffle(res.results[i]["out"]) for i in range(N_CORES)], axis=0
    )


# revision 35
# speedup vs baseline: 1.3545x; 1.0127x over previous
"""CARAFE kernel for Trainium2 (8 NeuronCores, batch-parallel), v2.

Reference computation per image:
  R = relu(conv1x1(x, w_compress, b_compress))          [48, 128, 128]
  E = conv3x3(R, w_encoder, b_encoder, pad=1)           [100, 128, 128]
  Y = softmax over k of E.reshape(4, 25, H, W)          (s, k, h, w)
  out[s,c,h,w] = sum_k Y[s,k,h,w] * xpad[c, h+dy, w+dx] (k=(dy,dx), 5x5, pad 2)
  pixel-shuffle: out_ref[s*16 + c//4, 2h + (c//2)%2, 2w + c%2] = out[s,c,h,w]

v2 design (vs v1):
  - bf16 everywhere: PE matmuls at 1 cyc/row (vs 4 for fp32), DVE
    tensor_tensor at 2x, all DMA bytes halved.  PSUM accumulation stays
    fp32, exp runs on fp32 PSUM logits.
  - The five dy-shifted pixel-major copies of x (needed because compute
    engines cannot shift partitions) are built on the HOST as a pure
    layout transform and shipped as one [128, 5*C*(W+4)] bf16 input --
    one contiguous DMA instead of 40 SBUF->SBUF copies with 272B
    descriptors (which dominated v1: 328K DMA packets).
  - Softmax normalization folded in per conv block: Z = sones @ F on PE,
    1/Z on DVE, broadcast back over the 25 taps via a second tiny matmul,
    one in-place multiply.  No F DRAM round-trip, no per-(s,w) epilogue.
  - Output leaves the device as raw [s, h, (c,w)] bf16 (4 contiguous 2MB
    DMAs); pixel-shuffle + fp32 cast happen on the host.
"""

import sys

import numpy as np

sys.path.insert(0, "/opt/trn_rl_repo")

import ml_dtypes

import concourse.bass as bass
import concourse.mybir as mybir
import concourse.tile as tile
from concourse import bacc

F32 = mybir.dt.float32
# fp16 (not bf16): the 25-term sequential accumulation in the patch sum
# needs the 10-bit mantissa — bf16's 7 bits put rel-err right at the 2e-2
# gate; fp16 lands at ~2e-3.  Values are O(1-5), far from fp16 range limits.
BF16 = mybir.dt.float16
BF_NP = np.float16

H = 128
W = 128
C = 64
M = 48  # compressed channels
S2 = 4  # scale_factor**2
K2 = 25  # k_up**2
SK = 100
HW = H * W
WPAD = W + 4  # w-padded pixel-major buffers
CW = C * WPAD  # 8448, free elems per (dy) plane
N_CORES = 8
NBLK = HW // 512  # 32 conv blocks of 512 pixels


def _ap(t, extra_off, dims):
    """Raw AP on a tile handle `t` with free-offset `extra_off` (elements)
    and explicit [step, count] dims (dims[0] is the partition dim)."""
    base = t[:]
    return bass.AP(tensor=base.tensor, offset=base.offset + extra_off, ap=dims)


class _Pool:
    """Manually scoped tile pool."""

    def __init__(self, tc, **kw):
        self._cm = tc.tile_pool(**kw)
        self.pool = self._cm.__enter__()
        self._n = 0

    def tile(self, *a, tag=None, **kw):
        self._n += 1
        t = tag or f"t{self._n}"
        return self.pool.tile(*a, tag=t, name=t, **kw)

    def close(self):
        self._cm.__exit__(None, None, None)


def build_program():
    nc = bacc.Bacc("TRN2", target_bir_lowering=False, debug=False)

    xc = nc.dram_tensor("xc", [C + 1, HW], BF16, kind="ExternalInput")
    xt5d = nc.dram_tensor("xt5", [128, 5 * CW], BF16, kind="ExternalInput")
    w1t = nc.dram_tensor("w1t", [C + 1, M], BF16, kind="ExternalInput")
    wet = nc.dram_tensor("wet", [113, 6 * SK], BF16, kind="ExternalInput")
    sones = nc.dram_tensor("sones", [SK, S2], BF16, kind="ExternalInput")
    sonesT = nc.dram_tensor("sonesT", [S2, SK], BF16, kind="ExternalInput")
    identd = nc.dram_tensor("ident", [SK, SK], BF16, kind="ExternalInput")
    onesr = nc.dram_tensor("onesr", [1, 130 * 130], BF16, kind="ExternalInput")
    out = nc.dram_tensor("out", [S2, H, C * W], BF16, kind="ExternalOutput")

    with tile.TileContext(nc) as tc:
        cp = _Pool(tc, name="consts", bufs=1)
        w1t_sb = cp.tile([C + 1, M], BF16)
        nc.sync.dma_start(w1t_sb[:], w1t.ap())
        wet_sb = cp.tile([113, 6 * SK], BF16)
        nc.sync.dma_start(wet_sb[:], wet.ap())
        sones_sb = cp.tile([SK, S2], BF16)
        nc.sync.dma_start(sones_sb[:], sones.ap())
        sonesT_sb = cp.tile([S2, SK], BF16)
        nc.sync.dma_start(sonesT_sb[:], sonesT.ap())
        ident_sb = cp.tile([SK, SK], BF16)
        nc.sync.dma_start(ident_sb[:], identd.ap())

        pp = _Pool(tc, name="persist", bufs=1)
        xt5 = pp.tile([128, 5 * CW], BF16)
        fr = pp.tile([128, SK * W], BF16)

        p2 = _Pool(tc, name="fnorm", bufs=1)
        f_norm = p2.tile([SK, HW], BF16)

        # ---- conv1x1 -> relu -> R_pad (full image, 1-halo borders) ----
        # Two copies of R on partitions 0..48 (A) and 64..112 (B, shifted one
        # w-column left) let conv3x3 contract taps (ty,0)+(ty,1) in a single
        # K=113 matmul: 6 matmuls per block instead of 9.
        RPF = 130 * 130
        p3 = _Pool(tc, name="rpad", bufs=1)
        r_pad = p3.tile([113, RPF], BF16)
        nc.gpsimd.memset(r_pad[:], 0.0)
        nc.sync.dma_start(
            _ap(r_pad, M * RPF, [[RPF, 1], [1, RPF]]), onesr.ap()
        )
        nc.sync.dma_start(
            _ap(r_pad, (M + 64) * RPF, [[RPF, 1], [1, RPF]]), onesr.ap()
        )

        p4 = _Pool(tc, name="xb", bufs=2)
        psA = _Pool(tc, name="psA", bufs=2, space="PSUM")
        CHUNK = 8  # conv1x1 blocks per x chunk-load
        for jc in range(NBLK // CHUNK):
            xb = p4.tile([C + 1, 512 * CHUNK], BF16, tag="xb")
            nc.sync.dma_start(
                xb[:], xc.ap()[:, jc * 512 * CHUNK : (jc + 1) * 512 * CHUNK]
            )
            for ji in range(CHUNK):
                j = jc * CHUNK + ji
                ps1 = psA.tile([M, 512], F32, tag="ps1")
                nc.tensor.matmul(
                    ps1[:],
                    w1t_sb[:],
                    xb[:, ji * 512 : (ji + 1) * 512],
                    start=True,
                    stop=True,
                )
                nc.scalar.activation(
                    _ap(
                        r_pad,
                        (1 + 4 * j) * 130 + 1,
                        [[RPF, M], [130, 4], [1, W]],
                    ),
                    ps1[:],
                    mybir.ActivationFunctionType.Relu,
                )
                nc.scalar.activation(
                    _ap(
                        r_pad,
                        64 * RPF + (1 + 4 * j) * 130,
                        [[RPF, M], [130, 4], [1, W]],
                    ),
                    ps1[:],
                    mybir.ActivationFunctionType.Relu,
                )
        psA.close()
        p4.close()

        # xt5 (10.8MB) is only needed by the patch phase; issuing it here
        # keeps the tiny conv1x1 input loads from queueing behind it.
        nc.sync.dma_start(xt5[:], xt5d.ap())

        # ---- conv3x3 -> exp -> normalized F (channel-major, bf16) ----
        psB = _Pool(tc, name="psB", bufs=4, space="PSUM")
        psC = _Pool(tc, name="psC", bufs=2, space="PSUM")
        psD = _Pool(tc, name="psD", bufs=2, space="PSUM")
        rzp = _Pool(tc, name="rz", bufs=2)
        # slots: 3 paired (taps (ty,0)+(ty,1), K=113) + 3 single (taps (ty,2), K=49)
        SLOTS = [(0, 113), (1, 113), (2, 113), (0, 49), (1, 49), (2, 49)]
        for j in range(NBLK):
            ps2 = psB.tile([SK, 512], F32, tag="ps2")
            for m, (ty, kk) in enumerate(SLOTS):
                off = (4 * j + ty) * 130 + (0 if kk == 113 else 2)
                nc.tensor.matmul(
                    ps2[:],
                    wet_sb[0:kk, m * SK : (m + 1) * SK],
                    _ap(r_pad, off, [[RPF, kk], [130, 4], [1, W]]),
                    start=(m == 0),
                    stop=(m == len(SLOTS) - 1),
                )
            fblk = f_norm[:, j * 512 : (j + 1) * 512]
            nc.scalar.activation(fblk, ps2[:], mybir.ActivationFunctionType.Exp)
            psz = psC.tile([S2, 512], F32, tag="psz")
            nc.tensor.matmul(psz[:], sones_sb[:], fblk, start=True, stop=True)
            rz32 = rzp.tile([S2, 512], F32, tag="rz32")
            nc.vector.reciprocal_approx_fast(rz32[:], psz[:])
            rz = rzp.tile([S2, 512], BF16, tag="rz")
            nc.scalar.copy(rz[:], rz32[:])
            zb = psD.tile([SK, 512], F32, tag="zb")
            nc.tensor.matmul(zb[:], sonesT_sb[:], rz[:], start=True, stop=True)
            nc.vector.tensor_mul(fblk, fblk, zb[:])
        rzp.close()
        psD.close()
        psC.close()
        psB.close()
        p3.close()

        # ---- F^T transposes -> FR [128(h), (sk, w)] ----
        # 8 per-column transposes share one PSUM bank, then a single strided
        # ScalarE copy moves all 8 columns into FR (vs 128 tiny copies).
        psF = _Pool(tc, name="psF", bufs=4, space="PSUM")
        FTG = 8
        for w0 in range(0, W, FTG):
            pst = psF.tile([128, FTG * SK], BF16, tag="pst")
            for wi in range(FTG):
                nc.tensor.transpose(
                    pst[:, wi * SK : (wi + 1) * SK],
                    _ap(f_norm, w0 + wi, [[HW, SK], [W, H]]),
                    ident_sb[:],
                )
            nc.scalar.copy(
                _ap(fr, w0, [[SK * W, 128], [1, FTG], [W, SK]]),
                _ap(pst, 0, [[FTG * SK, 128], [SK, FTG], [1, SK]]),
            )
        psF.close()
        p2.close()

        # ---- per-pixel patch sum on VectorE; out[s] = [128(h), (c, w)] ----
        pacc = _Pool(tc, name="acc", bufs=2)
        ptmp = _Pool(tc, name="tmp", bufs=2)

        def tap_aps(s, k, t):
            dy, dx = k // 5 - 2, k % 5 - 2
            in0 = _ap(
                xt5, (dy + 2) * CW + 2 + dx, [[5 * CW, 128], [WPAD, C], [1, W]]
            )
            in1 = _ap(fr, (s * K2 + k) * W, [[SK * W, 128], [0, C], [1, W]])
            dst3 = _ap(t, 0, [[C * W, 128], [W, C], [1, W]])
            return in0, in1, dst3

        for s in range(S2):
            acc = pacc.tile([128, C * W], BF16, tag="acc")
            for k in range(25):
                in0, in1, dst3 = tap_aps(s, k, acc)
                if k == 0:
                    nc.vector.tensor_mul(dst3, in0, in1)
                else:
                    tmp = ptmp.tile([128, C * W], BF16, tag="tmp")
                    _, _, t3 = tap_aps(s, k, tmp)
                    nc.vector.tensor_mul(t3, in0, in1)
                    nc.vector.tensor_add(acc[:], acc[:], tmp[:])
            nc.sync.dma_start(
                bass.AP(tensor=out, offset=s * H * C * W, ap=[[C * W, 128], [1, C * W]]),
                acc[:],
            )
        ptmp.close()
        pacc.close()
        pp.close()
        cp.close()
    nc.compile()
    return nc


def host_inputs(x_img, w_compress, b_compress, w_encoder, b_encoder):
    """Per-core input map for one image [C, H, W] (all bf16)."""
    x_img = np.asarray(x_img, np.float32)
    xc = np.concatenate(
        [x_img.reshape(C, HW), np.ones((1, HW), np.float32)], axis=0
    ).astype(BF_NP)
    # pixel-major, w-padded, 5 dy-shifted planes: xt5[h, dy, c, wp]
    #   = xpad[c, h + dy, wp]  (xpad has pad 2 on h and w)
    xpad = np.pad(x_img, ((0, 0), (2, 2), (2, 2))).astype(BF_NP)
    xt5 = np.stack([xpad[:, dy : dy + H, :] for dy in range(5)], axis=0)
    xt5 = np.ascontiguousarray(xt5.transpose(2, 0, 1, 3)).reshape(128, 5 * CW)
    w1t = np.concatenate(
        [w_compress[:, :, 0, 0].T, b_compress[None, :]], axis=0
    ).astype(BF_NP)
    # paired layout: slots 0-2 = taps (ty,0) on rows 0..47 + (ty,1) on rows
    # 64..111; slots 3-5 = single taps (ty,2).  Bias rides the all-ones rows
    # (48 for A, 112 for B) on the center tap (1,1) = slot 1's B half.
    wetm = np.zeros((113, 6, SK), np.float32)
    for ty in range(3):
        wetm[:M, ty, :] = w_encoder[:, :, ty, 0].T
        wetm[64 : 64 + M, ty, :] = w_encoder[:, :, ty, 1].T
        wetm[:M, 3 + ty, :] = w_encoder[:, :, ty, 2].T
    wetm[112, 1, :] = b_encoder
    son = np.zeros((SK, S2), np.float32)
    for s in range(S2):
        son[s * K2 : (s + 1) * K2, s] = 1.0
    return {
        "xc": xc,
        "xt5": xt5,
        "w1t": w1t,
        "wet": wetm.reshape(113, 6 * SK).astype(BF_NP),
        "sones": son.astype(BF_NP),
        "sonesT": np.ascontiguousarray(son.T).astype(BF_NP),
        "ident": np.eye(SK, dtype=BF_NP),
        "onesr": np.ones((1, 130 * 130), BF_NP),
    }


def _unshuffle(dev_out):
    """[S2, H, C*W] bf16 -> [64, 256, 256] fp32 pixel-shuffled output."""
    a = np.asarray(dev_out).reshape(S2, H, 16, 2, 2, W)  # s, h, c4, c2, c1, w
    a = a.transpose(0, 2, 1, 3, 5, 4)  # s, c4, h, c2, w, c1
    return np.ascontiguousarray(a).reshape(C, 2 * H, 2 * W).astype(np.float32)


_CACHE = {}


def kernel(x, w_compress, b_compress, w_encoder, b_encoder):
    x = np.asarray(x, np.float32)
    if "nc" not in _CACHE:
        _CACHE["nc"] = build_program()
    nc = _CACHE["nc"]
    in_maps = [
        host_inputs(
            x[i],
            np.asarray(w_compress, np.float32),
            np.asarray(b_compress, np.float32),
            np.asarray(w_encoder, np.float32),
            np.asarray(b_encoder, np.float32),
        )
        for i in range(N_CORES)
    ]
    from concourse.bass_utils import run_bass_kernel_spmd

    res = run_bass_kernel_spmd(nc, in_maps, core_ids=list(range(N_CORES)))
    return np.stack(
        [_unshuffle(res.results[i]["out"]) for i in range(N_CORES)], axis=0
    )


# revision 36
# speedup vs baseline: 1.3935x; 1.0288x over previous
"""CARAFE kernel for Trainium2 (8 NeuronCores, batch-parallel), v2.

Reference computation per image:
  R = relu(conv1x1(x, w_compress, b_compress))          [48, 128, 128]
  E = conv3x3(R, w_encoder, b_encoder, pad=1)           [100, 128, 128]
  Y = softmax over k of E.reshape(4, 25, H, W)          (s, k, h, w)
  out[s,c,h,w] = sum_k Y[s,k,h,w] * xpad[c, h+dy, w+dx] (k=(dy,dx), 5x5, pad 2)
  pixel-shuffle: out_ref[s*16 + c//4, 2h + (c//2)%2, 2w + c%2] = out[s,c,h,w]

v2 design (vs v1):
  - bf16 everywhere: PE matmuls at 1 cyc/row (vs 4 for fp32), DVE
    tensor_tensor at 2x, all DMA bytes halved.  PSUM accumulation stays
    fp32, exp runs on fp32 PSUM logits.
  - The five dy-shifted pixel-major copies of x (needed because compute
    engines cannot shift partitions) are built on the HOST as a pure
    layout transform and shipped as one [128, 5*C*(W+4)] bf16 input --
    one contiguous DMA instead of 40 SBUF->SBUF copies with 272B
    descriptors (which dominated v1: 328K DMA packets).
  - Softmax normalization folded in per conv block: Z = sones @ F on PE,
    1/Z on DVE, broadcast back over the 25 taps via a second tiny matmul,
    one in-place multiply.  No F DRAM round-trip, no per-(s,w) epilogue.
  - Output leaves the device as raw [s, h, (c,w)] bf16 (4 contiguous 2MB
    DMAs); pixel-shuffle + fp32 cast happen on the host.
"""

import sys

import numpy as np

sys.path.insert(0, "/opt/trn_rl_repo")

import ml_dtypes

import concourse.bass as bass
import concourse.mybir as mybir
import concourse.tile as tile
from concourse import bacc

F32 = mybir.dt.float32
# fp16 (not bf16): the 25-term sequential accumulation in the patch sum
# needs the 10-bit mantissa — bf16's 7 bits put rel-err right at the 2e-2
# gate; fp16 lands at ~2e-3.  Values are O(1-5), far from fp16 range limits.
BF16 = mybir.dt.float16
BF_NP = np.float16

H = 128
W = 128
C = 64
M = 48  # compressed channels
S2 = 4  # scale_factor**2
K2 = 25  # k_up**2
SK = 100
HW = H * W
WPAD = W + 4  # w-padded pixel-major buffers
CW = C * WPAD  # 8448, free elems per (dy) plane
N_CORES = 8
NBLK = HW // 512  # 32 conv blocks of 512 pixels


def _ap(t, extra_off, dims):
    """Raw AP on a tile handle `t` with free-offset `extra_off` (elements)
    and explicit [step, count] dims (dims[0] is the partition dim)."""
    base = t[:]
    return bass.AP(tensor=base.tensor, offset=base.offset + extra_off, ap=dims)


class _Pool:
    """Manually scoped tile pool."""

    def __init__(self, tc, **kw):
        self._cm = tc.tile_pool(**kw)
        self.pool = self._cm.__enter__()
        self._n = 0

    def tile(self, *a, tag=None, **kw):
        self._n += 1
        t = tag or f"t{self._n}"
        return self.pool.tile(*a, tag=t, name=t, **kw)

    def close(self):
        self._cm.__exit__(None, None, None)


def build_program():
    nc = bacc.Bacc("TRN2", target_bir_lowering=False, debug=False)

    xc = nc.dram_tensor("xc", [C + 1, HW], BF16, kind="ExternalInput")
    xt5d = nc.dram_tensor("xt5", [128, 5 * CW], BF16, kind="ExternalInput")
    w1t = nc.dram_tensor("w1t", [C + 1, M], BF16, kind="ExternalInput")
    wet = nc.dram_tensor("wet", [113, 6 * SK], BF16, kind="ExternalInput")
    sones = nc.dram_tensor("sones", [SK, S2], BF16, kind="ExternalInput")
    sonesT = nc.dram_tensor("sonesT", [S2, SK], BF16, kind="ExternalInput")
    identd = nc.dram_tensor("ident", [SK, SK], BF16, kind="ExternalInput")
    onesr = nc.dram_tensor("onesr", [1, 130 * 130], BF16, kind="ExternalInput")
    out = nc.dram_tensor("out", [S2, H, C * W], BF16, kind="ExternalOutput")

    with tile.TileContext(nc) as tc:
        cp = _Pool(tc, name="consts", bufs=1)
        w1t_sb = cp.tile([C + 1, M], BF16)
        nc.sync.dma_start(w1t_sb[:], w1t.ap())
        wet_sb = cp.tile([113, 6 * SK], BF16)
        nc.sync.dma_start(wet_sb[:], wet.ap())
        sones_sb = cp.tile([SK, S2], BF16)
        nc.sync.dma_start(sones_sb[:], sones.ap())
        sonesT_sb = cp.tile([S2, SK], BF16)
        nc.sync.dma_start(sonesT_sb[:], sonesT.ap())
        ident_sb = cp.tile([SK, SK], BF16)
        nc.sync.dma_start(ident_sb[:], identd.ap())

        pp = _Pool(tc, name="persist", bufs=1)
        xt5 = pp.tile([128, 5 * CW], BF16)
        fr = pp.tile([128, SK * W], BF16)

        p2 = _Pool(tc, name="fnorm", bufs=1)
        f_norm = p2.tile([SK, HW], BF16)

        # ---- conv1x1 -> relu -> R_pad (full image, 1-halo borders) ----
        # Two copies of R on partitions 0..48 (A) and 64..112 (B, shifted one
        # w-column left) let conv3x3 contract taps (ty,0)+(ty,1) in a single
        # K=113 matmul: 6 matmuls per block instead of 9.
        RPF = 130 * 130
        p3 = _Pool(tc, name="rpad", bufs=1)
        r_pad = p3.tile([113, RPF], BF16)
        nc.gpsimd.memset(r_pad[:], 0.0)
        nc.sync.dma_start(
            _ap(r_pad, M * RPF, [[RPF, 1], [1, RPF]]), onesr.ap()
        )
        nc.sync.dma_start(
            _ap(r_pad, (M + 64) * RPF, [[RPF, 1], [1, RPF]]), onesr.ap()
        )

        p4 = _Pool(tc, name="xb", bufs=2)
        psA = _Pool(tc, name="psA", bufs=2, space="PSUM")
        CHUNK = 8  # conv1x1 blocks per x chunk-load
        for jc in range(NBLK // CHUNK):
            xb = p4.tile([C + 1, 512 * CHUNK], BF16, tag="xb")
            nc.sync.dma_start(
                xb[:], xc.ap()[:, jc * 512 * CHUNK : (jc + 1) * 512 * CHUNK]
            )
            for ji in range(CHUNK):
                j = jc * CHUNK + ji
                ps1 = psA.tile([M, 512], F32, tag="ps1")
                nc.tensor.matmul(
                    ps1[:],
                    w1t_sb[:],
                    xb[:, ji * 512 : (ji + 1) * 512],
                    start=True,
                    stop=True,
                )
                nc.scalar.activation(
                    _ap(
                        r_pad,
                        (1 + 4 * j) * 130 + 1,
                        [[RPF, M], [130, 4], [1, W]],
                    ),
                    ps1[:],
                    mybir.ActivationFunctionType.Relu,
                )
                nc.scalar.activation(
                    _ap(
                        r_pad,
                        64 * RPF + (1 + 4 * j) * 130,
                        [[RPF, M], [130, 4], [1, W]],
                    ),
                    ps1[:],
                    mybir.ActivationFunctionType.Relu,
                )
        psA.close()
        p4.close()

        # xt5 (10.8MB) is only needed by the patch phase; issuing it here
        # keeps the tiny conv1x1 input loads from queueing behind it.
        nc.sync.dma_start(xt5[:], xt5d.ap())

        # ---- conv3x3 -> exp -> normalized F -> F^T, w-major blocks ----
        # Each block covers all 128 h rows x 4 w columns, so the per-column
        # F^T transposes interleave with the conv instead of trailing it.
        psB = _Pool(tc, name="psB", bufs=3, space="PSUM")
        psC = _Pool(tc, name="psC", bufs=2, space="PSUM")
        psD = _Pool(tc, name="psD", bufs=1, space="PSUM")
        psF = _Pool(tc, name="psF", bufs=2, space="PSUM")
        rzp = _Pool(tc, name="rz", bufs=2)
        # slots: 3 paired (taps (ty,0)+(ty,1), K=113) + 3 single (taps (ty,2), K=49)
        SLOTS = [(0, 113), (1, 113), (2, 113), (0, 49), (1, 49), (2, 49)]
        FTG = 8

        def fblk_ap(wb, t=None):
            return _ap(f_norm, 4 * wb, [[HW, SK], [W, H], [1, 4]])

        for wb in range(W // 4):
            ps2 = psB.tile([SK, 512], F32, tag="ps2")
            for m, (ty, kk) in enumerate(SLOTS):
                off = ty * 130 + (0 if kk == 113 else 2) + 4 * wb
                nc.tensor.matmul(
                    ps2[:],
                    wet_sb[0:kk, m * SK : (m + 1) * SK],
                    _ap(r_pad, off, [[RPF, kk], [130, H], [1, 4]]),
                    start=(m == 0),
                    stop=(m == len(SLOTS) - 1),
                )
            ps2v = _ap(ps2, 0, [[512, SK], [4, H], [1, 4]])
            nc.scalar.activation(
                fblk_ap(wb), ps2v, mybir.ActivationFunctionType.Exp
            )
            psz = psC.tile([S2, 512], F32, tag="psz")
            nc.tensor.matmul(psz[:], sones_sb[:], fblk_ap(wb), start=True, stop=True)
            rz32 = rzp.tile([S2, 512], F32, tag="rz32")
            nc.vector.reciprocal_approx_fast(rz32[:], psz[:])
            rz = rzp.tile([S2, 512], BF16, tag="rz")
            nc.scalar.copy(rz[:], rz32[:])
            zb = psD.tile([SK, 512], F32, tag="zb")
            nc.tensor.matmul(zb[:], sonesT_sb[:], rz[:], start=True, stop=True)
            nc.vector.tensor_mul(
                fblk_ap(wb), fblk_ap(wb), _ap(zb, 0, [[512, SK], [4, H], [1, 4]])
            )
            # every 2 blocks: transpose the 8 finished columns into FR
            if wb % 2 == 1:
                w0 = 4 * (wb - 1)
                pst = psF.tile([128, FTG * SK], BF16, tag="pst")
                for wi in range(FTG):
                    nc.tensor.transpose(
                        pst[:, wi * SK : (wi + 1) * SK],
                        _ap(f_norm, w0 + wi, [[HW, SK], [W, H]]),
                        ident_sb[:],
                    )
                nc.scalar.copy(
                    _ap(fr, w0, [[SK * W, 128], [1, FTG], [W, SK]]),
                    _ap(pst, 0, [[FTG * SK, 128], [SK, FTG], [1, SK]]),
                )
        rzp.close()
        psF.close()
        psD.close()
        psC.close()
        psB.close()
        p3.close()
        p2.close()

        # ---- per-pixel patch sum on VectorE; out[s] = [128(h), (c, w)] ----
        pacc = _Pool(tc, name="acc", bufs=2)
        ptmp = _Pool(tc, name="tmp", bufs=2)

        def tap_aps(s, k, t):
            dy, dx = k // 5 - 2, k % 5 - 2
            in0 = _ap(
                xt5, (dy + 2) * CW + 2 + dx, [[5 * CW, 128], [WPAD, C], [1, W]]
            )
            in1 = _ap(fr, (s * K2 + k) * W, [[SK * W, 128], [0, C], [1, W]])
            dst3 = _ap(t, 0, [[C * W, 128], [W, C], [1, W]])
            return in0, in1, dst3

        for s in range(S2):
            acc = pacc.tile([128, C * W], BF16, tag="acc")
            for k in range(25):
                in0, in1, dst3 = tap_aps(s, k, acc)
                if k == 0:
                    nc.vector.tensor_mul(dst3, in0, in1)
                else:
                    tmp = ptmp.tile([128, C * W], BF16, tag="tmp")
                    _, _, t3 = tap_aps(s, k, tmp)
                    nc.vector.tensor_mul(t3, in0, in1)
                    nc.vector.tensor_add(acc[:], acc[:], tmp[:])
            nc.sync.dma_start(
                bass.AP(tensor=out, offset=s * H * C * W, ap=[[C * W, 128], [1, C * W]]),
                acc[:],
            )
        ptmp.close()
        pacc.close()
        pp.close()
        cp.close()
    nc.compile()
    return nc


def host_inputs(x_img, w_compress, b_compress, w_encoder, b_encoder):
    """Per-core input map for one image [C, H, W] (all bf16)."""
    x_img = np.asarray(x_img, np.float32)
    xc = np.concatenate(
        [x_img.reshape(C, HW), np.ones((1, HW), np.float32)], axis=0
    ).astype(BF_NP)
    # pixel-major, w-padded, 5 dy-shifted planes: xt5[h, dy, c, wp]
    #   = xpad[c, h + dy, wp]  (xpad has pad 2 on h and w)
    xpad = np.pad(x_img, ((0, 0), (2, 2), (2, 2))).astype(BF_NP)
    xt5 = np.stack([xpad[:, dy : dy + H, :] for dy in range(5)], axis=0)
    xt5 = np.ascontiguousarray(xt5.transpose(2, 0, 1, 3)).reshape(128, 5 * CW)
    w1t = np.concatenate(
        [w_compress[:, :, 0, 0].T, b_compress[None, :]], axis=0
    ).astype(BF_NP)
    # paired layout: slots 0-2 = taps (ty,0) on rows 0..47 + (ty,1) on rows
    # 64..111; slots 3-5 = single taps (ty,2).  Bias rides the all-ones rows
    # (48 for A, 112 for B) on the center tap (1,1) = slot 1's B half.
    wetm = np.zeros((113, 6, SK), np.float32)
    for ty in range(3):
        wetm[:M, ty, :] = w_encoder[:, :, ty, 0].T
        wetm[64 : 64 + M, ty, :] = w_encoder[:, :, ty, 1].T
        wetm[:M, 3 + ty, :] = w_encoder[:, :, ty, 2].T
    wetm[112, 1, :] = b_encoder
    son = np.zeros((SK, S2), np.float32)
    for s in range(S2):
        son[s * K2 : (s + 1) * K2, s] = 1.0
    return {
        "xc": xc,
        "xt5": xt5,
        "w1t": w1t,
        "wet": wetm.reshape(113, 6 * SK).astype(BF_NP),
        "sones": son.astype(BF_NP),
        "sonesT": np.ascontiguousarray(son.T).astype(BF_NP),
        "ident": np.eye(SK, dtype=BF_NP),
        "onesr": np.ones((1, 130 * 130), BF_NP),
    }


def _unshuffle(dev_out):
    """[S2, H, C*W] bf16 -> [64, 256, 256] fp32 pixel-shuffled output."""
    a = np.asarray(dev_out).reshape(S2, H, 16, 2, 2, W)  # s, h, c4, c2, c1, w
    a = a.transpose(0, 2, 1, 3, 5, 4)  # s, c4, h, c2, w, c1
    return np.ascontiguousarray(a).reshape(C, 2 * H, 2 * W).astype(np.float32)


_CACHE = {}


def kernel(x, w_compress, b_compress, w_encoder, b_encoder):
    x = np.asarray(x, np.float32)
    if "nc" not in _CACHE:
        _CACHE["nc"] = build_program()
    nc = _CACHE["nc"]
    in_maps = [
        host_inputs(
            x[i],
            np.asarray(w_compress, np.float32),
            np.asarray(b_compress, np.float32),
            np.asarray(w_encoder, np.float32),
            np.asarray(b_encoder, np.float32),
        )
        for i in range(N_CORES)
    ]
    from concourse.bass_utils import run_bass_kernel_spmd

    res = run_bass_kernel_spmd(nc, in_maps, core_ids=list(range(N_CORES)))
    return np.stack(
        [_unshuffle(res.results[i]["out"]) for i in range(N_CORES)], axis=0
    )
